# revision 3
# baseline (speedup 1.0000x reference)
"""Mimi-style GQA attention (RoPE + 250-wide sliding causal window) on 8 TRN2 NeuronCores.

Sharding: core c handles batch b=c//4 and KV-head group g=c%4 (4 query heads +
1 KV head). Wq/Wk/Wv column-sharded, Wo row-sharded along the head dim; each
core emits a partial [S, D] output (fp16); host sums the 4 partials per batch.

v3 design (vs the 118.7us v2 baseline):
  * All DRAM inputs host-prearranged as [128, X] partition-contiguous blocks:
    every load runs with >=2KB descriptors (the v2 rearranged loads moved at
    ~37GB/s on 512B descriptors, starving the first 25us).
  * hsT streamed strip-major on both HWDGE queues; 16 HAM warm-up matmuls
    cover the stream lead-in so projections start warm at ~8us.
  * Head dims stored pair-interleaved (sigma order [0,32,1,33,...]): RoPE
    rotate-half becomes a within-quadrant partition swap - one vector
    stream_shuffle + two multiplies + add, no PE perm matmuls.
  * The sliding-window mask is applied on the PE as additive -30000 matmuls
    (identity stationary, masked columns only) accumulated into the score
    PSUM before exp: the vector engine leaves the scores->exp->PV chain
    entirely, and the et tiles need no post-exp masking.
  * PV stationaries are [V|ones] / [ones|V] so the softmax denominator lands
    broadcast on complementary partitions for free; normalization is one
    reciprocal_approx_fast + two partition-shift DMAs + two lane-aligned
    multiplies per (strip, head-pair).
  * Two-phase schedule with one unified 4-buf PSUM pool (8 banks): dense
    projection+rope phase, then per-strip scores/exp, PV bursts, norms, with
    the previous strip's output projections as PE filler; channel-split tail.
  * Engine balance: scalar = exp + PSUM staging copies; vector = rope +
    reciprocal + norm + half the output copies; gpsimd = cos-path multiplies
    and bulk DMA dispatch only.
"""
import os
import sys

for _p in ("/opt/trn_rl_repo", "/root/.axon_site/_ro/trn_rl_repo"):
    if os.path.isdir(_p) and _p not in sys.path:
        sys.path.append(_p)

import numpy as np
import concourse.bass as bass
import concourse.mybir as mybir
import concourse.tile as tile
from concourse import bacc
from concourse.bass_utils import run_bass_kernel_spmd
from concourse.masks import make_identity

F32 = mybir.dt.float32
F16 = mybir.dt.float16
AF = mybir.ActivationFunctionType
OP = mybir.AluOpType

B, S, D = 2, 2048, 1024
H, HK, HD = 16, 4, 64
WINDOW = 250
SCALE = 1.0 / np.sqrt(HD)
THETA = 10000.0
NKT = S // 128          # 16 k-tiles
NST = S // 512          # 4 q-strips
WIN = 384               # padded per-k-tile q-window

SHUF_MASK = []
for _i in range(16):
    SHUF_MASK += [2 * _i + 1, 2 * _i]


def _pv_pieces(s):
    """PV pieces for q-strip s: list of (kt, c0, c1) window-column ranges."""
    out = []
    for kt in range(max(0, 4 * s - 2), min(NKT - 1, 4 * s + 3) + 1):
        j0 = 128 * kt
        w = min(WIN, S - j0)
        c_lo = max(0, 512 * s - j0)
        c_hi = min(w, 512 * (s + 1) - j0)
        if c_lo < c_hi:
            out.append((kt, c_lo, c_hi))
    return out


def _build():
    nc = bacc.Bacc(None, target_bir_lowering=False)

    hsta = [nc.declare_dram_parameter(f"hsTa{s}", [128, 4, 512], F16, isOutput=False)
            for s in range(NST)]
    hstb = [nc.declare_dram_parameter(f"hsTb{s}", [128, 4, 512], F16, isOutput=False)
            for s in range(NST)]
    wq = nc.declare_dram_parameter("wqT", [128, 8, 256], F16, isOutput=False)
    wkv = nc.declare_dram_parameter("wkvT", [128, 8, 128], F16, isOutput=False)
    wo = nc.declare_dram_parameter("woT", [128, 2, D], F16, isOutput=False)
    cosd = nc.declare_dram_parameter("cos2", [128, S], F16, isOutput=False)
    sind = nc.declare_dram_parameter("sinS2", [128, S], F16, isOutput=False)
    maskd = nc.declare_dram_parameter("bandmask", [128, WIN], F16, isOutput=False)
    o_part = nc.declare_dram_parameter("o_part", [S, D], F16, isOutput=True)

    with tile.TileContext(nc) as tc:
        with (
            tc.tile_pool(name="persist", bufs=1) as pp,
            tc.tile_pool(name="work", bufs=8) as wkp,
            tc.tile_pool(name="ework", bufs=8) as ep,
            tc.tile_pool(name="norm", bufs=2) as npl,
            tc.tile_pool(name="ost", bufs=3) as opl,
        ):
            # ---- input DMAs (host-prearranged [128, X]: big descriptors) ----
            # scalar HWDGE queue: weights only (small, early, never blocks).
            wq_sb = pp.tile([128, 8, 256], F16, tag="wq")
            wkv_sb = pp.tile([128, 8, 128], F16, tag="wkv")
            nc.scalar.dma_start(out=wq_sb[:, 0:4, :], in_=wq[:, 0:4, :])
            nc.scalar.dma_start(out=wkv_sb, in_=wkv[:, :, :])
            nc.scalar.dma_start(out=wq_sb[:, 4:8, :], in_=wq[:, 4:8, :])
            # sync HWDGE queue: the hsT stream, strip-major; strip 0 in
            # quarter-granularity so the first matmuls gate on ~256KB.
            hta, htb = [], []
            htb_pend = []
            for s in range(NST):
                ta = pp.tile([128, 4, 512], F16, tag=f"hta{s}")
                tb = pp.tile([128, 4, 512], F16, tag=f"htb{s}")
                if s == 0:
                    nc.sync.dma_start(out=ta[:, 0:2, :], in_=hsta[s][:, 0:2, :])
                    nc.sync.dma_start(out=ta[:, 2:4, :], in_=hsta[s][:, 2:4, :])
                    nc.sync.dma_start(out=tb[:, 0:2, :], in_=hstb[s][:, 0:2, :])
                    nc.sync.dma_start(out=tb[:, 2:4, :], in_=hstb[s][:, 2:4, :])
                elif s == 1:
                    nc.sync.dma_start(out=ta, in_=hsta[s][:, :, :])
                    nc.sync.dma_start(out=tb, in_=hstb[s][:, :, :])
                else:
                    # lower halves dispatched mid-phase-1 on the scalar queue
                    nc.sync.dma_start(out=tb, in_=hstb[s][:, :, :])
                hta.append(ta)
                htb.append(tb)
            mask_sb = pp.tile([128, WIN], F16, tag="mask")
            nc.scalar.dma_start(out=mask_sb, in_=maskd[:, :])
            # gpsimd SWDGE queue: aux tensors not needed before ~4us.
            cos_sb = pp.tile([128, S], F16, tag="cos")
            sin_sb = pp.tile([128, S], F16, tag="sin")
            nc.gpsimd.dma_start(out=cos_sb, in_=cosd[:, :])
            nc.gpsimd.dma_start(out=sin_sb, in_=sind[:, :])
            wo_sb = pp.tile([128, 2, D], F16, tag="wo")
            nc.gpsimd.dma_start(out=wo_sb, in_=wo[:, :, :])

            warm = pp.tile([128, 512], F16, tag="warm")
            nc.vector.memset(warm, 0.0)
            ident = pp.tile([64, 64], F16, tag="ident")
            make_identity(nc, ident)
            ident128 = pp.tile([128, 128], F16, tag="id128")
            make_identity(nc, ident128)
            vload = pp.tile([128, NKT, 192], F16, tag="vload")
            nc.gpsimd.memset(vload, 1.0)

            kdup = pp.tile([128, S], F16, tag="kdup")
            qT = [pp.tile([128, S], F16, tag=f"qT{m}", name=f"qT{m}") for m in range(2)]
            aT = [pp.tile([128, S], F16, tag=f"aT{ch}", name=f"aT{ch}") for ch in range(2)]

            with tc.tile_pool(name="pu", bufs=4, space="PSUM") as pu:
                p_sc = p_pv = pu
                expm = {}

                def emit_kv(s):
                    """KV projection + K-RoPE + V transposes. Decoupled from
                    the Q path: scalar only stages rawkv; V^T copies land on
                    vector where they have strip-scale slack."""
                    sl = bass.ts(s, 512)
                    tk = pu.tile([128, 2, 512], F32, tag="pv", name=f"tk{s}")
                    for dt in range(8):
                        h = hta[s][:, dt, :] if dt < 4 else htb[s][:, dt - 4, :]
                        nc.tensor.matmul(tk[:, 0, :], wkv_sb[:, dt, :], h,
                                         start=(dt == 0), stop=(dt == 7))
                    rawkv = wkp.tile([128, 512], F16, tag="rawkv")
                    nc.scalar.copy(rawkv, tk[:, 0, :])
                    rotk = wkp.tile([128, 512], F16, tag="rot")
                    nc.vector.stream_shuffle(rotk[64:128, :], rawkv[64:128, :],
                                             SHUF_MASK)
                    t1k = wkp.tile([128, 512], F16, tag="t1")
                    nc.vector.tensor_tensor(out=t1k[64:128, :], in0=rotk[64:128, :],
                                            in1=sin_sb[64:128, sl], op=OP.mult)
                    t2k = wkp.tile([128, 512], F16, tag="t2")
                    nc.gpsimd.tensor_tensor(out=t2k[64:128, :], in0=rawkv[64:128, :],
                                            in1=cos_sb[64:128, sl], op=OP.mult)
                    nc.vector.tensor_tensor(out=kdup[64:128, sl], in0=t1k[64:128, :],
                                            in1=t2k[64:128, :], op=OP.add)
                    nc.scalar.dma_start(out=kdup[0:64, sl], in_=kdup[64:128, sl])
                    for j in range(4):
                        kt = 4 * s + j
                        tr = tk[:, 1, 32 * j:32 * j + 32].bitcast(F16)
                        nc.tensor.transpose(tr, rawkv[0:64, bass.ts(j, 128)], ident)
                        nc.scalar.copy(vload[:, kt, 64:128], tr)

                def emit_q_mm(s):
                    """Q projection matmuls; PSUM drained by vector copies
                    emitted immediately (they are ready as soon as the
                    accumulation closes, so no FIFO head-of-line risk)."""
                    tq = pu.tile([128, 2, 512], F32, tag="pv", name=f"tq{s}")
                    for dt in range(4):
                        nc.tensor.matmul(tq[:, 0, :], wq_sb[:, dt, 0:128],
                                         hta[s][:, dt, :], start=(dt == 0), stop=False)
                        nc.tensor.matmul(tq[:, 1, :], wq_sb[:, dt, 128:256],
                                         hta[s][:, dt, :], start=(dt == 0), stop=False)
                    for dt in range(4, 8):
                        nc.tensor.matmul(tq[:, 0, :], wq_sb[:, dt, 0:128],
                                         htb[s][:, dt - 4, :], start=False, stop=(dt == 7))
                        nc.tensor.matmul(tq[:, 1, :], wq_sb[:, dt, 128:256],
                                         htb[s][:, dt - 4, :], start=False, stop=(dt == 7))
                    raws = []
                    for m in range(2):
                        raw = wkp.tile([128, 512], F16, tag=f"raw{m}")
                        nc.scalar.copy(raw, tq[:, m, :])
                        raws.append(raw)
                    return raws

                def emit_rope_q(s, raws):
                    sl = bass.ts(s, 512)
                    for m in range(2):
                        raw = raws[m]
                        rot = wkp.tile([128, 512], F16, tag="rot")
                        nc.vector.stream_shuffle(rot, raw, SHUF_MASK)
                        t1 = wkp.tile([128, 512], F16, tag="t1")
                        nc.vector.tensor_tensor(out=t1, in0=rot, in1=sin_sb[:, sl],
                                                op=OP.mult)
                        t2 = wkp.tile([128, 512], F16, tag="t2")
                        nc.gpsimd.tensor_tensor(out=t2, in0=raw,
                                                in1=cos_sb[:, sl], op=OP.mult)
                        nc.vector.tensor_tensor(out=qT[m][:, sl], in0=t1, in1=t2,
                                                op=OP.add)

                def emit_scores(kt, p):
                    j0 = 128 * kt
                    w = min(WIN, S - j0)
                    if p == 0:
                        expm[kt] = ep.tile([128, 4, WIN], F16, tag="e",
                                           name=f"e{kt}")
                    et4 = expm[kt]
                    sc = pu.tile([128, 2, 512], F32, tag="pv", name=f"sc{kt}_{p}")
                    for hh in range(2):
                        half = hh * 64
                        nc.tensor.matmul(
                            sc[:, hh, 0:w],
                            kdup[half:half + 64, bass.ts(kt, 128)],
                            qT[p][half:half + 64, j0:j0 + w],
                            start=True, stop=False)
                    # additive band mask (-30000 outside the window) applied
                    # on the PE: identity stationary, masked columns only.
                    if w == WIN:
                        for hh in range(2):
                            nc.tensor.matmul(sc[:, hh, 0:136], ident128,
                                             mask_sb[:, 0:136],
                                             start=False, stop=False)
                            nc.tensor.matmul(sc[:, hh, 248:384], ident128,
                                             mask_sb[:, 248:384],
                                             start=False, stop=True)
                    else:
                        for hh in range(2):
                            nc.tensor.matmul(sc[:, hh, 0:w], ident128,
                                             mask_sb[:, 0:w],
                                             start=False, stop=True)
                    nc.scalar.activation(et4[:, 2 * p:2 * p + 2, 0:w],
                                         sc[:, :, 0:w], AF.Exp, scale=float(SCALE))

                def emit_pv(s, hp):
                    pv = pu.tile([128, 2, 512], F32, tag="pv", name=f"pv{s}{hp}")
                    pieces = _pv_pieces(s)
                    subs = []
                    cw = 0
                    for kt, c0, c1 in pieces:
                        b = 128 * kt + c0 - 512 * s
                        e = b + (c1 - c0)
                        if b < cw:
                            subs.append((kt, c0, c0 + min(cw, e) - b, b))
                        if e > cw:
                            subs.append((kt, c0 + (max(b, cw) - b), c1, max(b, cw)))
                            cw = e
                    for i, (kt, c0, c1, b) in enumerate(subs):
                        first, last = (i == 0), (i == len(subs) - 1)
                        et4 = expm[kt]
                        nc.tensor.matmul(pv[:, 0, b:b + (c1 - c0)],
                                         vload[:, kt, 64:192],
                                         et4[:, 2 * hp, c0:c1],
                                         start=first, stop=last)
                        nc.tensor.matmul(pv[:, 1, b:b + (c1 - c0)],
                                         vload[:, kt, 0:128],
                                         et4[:, 2 * hp + 1, c0:c1],
                                         start=first, stop=last)
                    return pv

                def emit_norm(s, hp, pv):
                    sl = bass.ts(s, 512)
                    rcp = npl.tile([128, 2, 512], F32, tag="rcp")
                    nc.vector.reciprocal_approx_fast(out=rcp, in_=pv)
                    rcpS = npl.tile([128, 2, 512], F32, tag="rcpS")
                    nc.sync.dma_start(out=rcpS[0:64, 0, :], in_=rcp[64:128, 0, :])
                    nc.sync.dma_start(out=rcpS[64:128, 1, :], in_=rcp[0:64, 1, :])
                    nc.vector.tensor_tensor(out=aT[hp][0:64, sl],
                                            in0=pv[0:64, 0, :],
                                            in1=rcpS[0:64, 0, :], op=OP.mult)
                    nc.vector.tensor_tensor(out=aT[hp][64:128, sl],
                                            in0=pv[64:128, 1, :],
                                            in1=rcpS[64:128, 1, :], op=OP.mult)

                def emit_pso(st, copy_eng=None):
                    psos = pu.tile([128, 2, 512], F32, tag="pv", name=f"pso{st}")
                    for ch in range(2):
                        for dx in range(2):
                            nc.tensor.matmul(psos[:, dx, :],
                                             aT[ch][:, bass.ts(st, 128)],
                                             wo_sb[:, ch, bass.ts(dx, 512)],
                                             start=(ch == 0), stop=(ch == 1))
                    ost = opl.tile([128, 1024], F16, tag="o")
                    eng = copy_eng or ("scalar" if st % 2 == 0 else "vector")
                    if eng == "scalar":
                        nc.scalar.copy(ost, psos)
                    else:
                        nc.vector.tensor_scalar_mul(ost, psos, 1.0)
                    nc.gpsimd.dma_start(out=o_part[bass.ts(st, 128), :], in_=ost)

                # ---- master schedule ----
                wps = pu.tile([128, 2, 512], F32, tag="pv", name="warm")
                for _w in range(16):
                    nc.tensor.matmul(wps[:, _w % 2, :], warm[:, 0:128], warm,
                                     start=True, stop=True)

                # phase 1: dense projections + rope, strip-streamed with DMA
                rq = {}
                for s in range(NST):
                    if s == 2:
                        nc.scalar.dma_start(out=hta[2], in_=hsta[2][:, :, :])
                        nc.scalar.dma_start(out=hta[3], in_=hsta[3][:, :, :])
                    rq[s] = emit_q_mm(s)
                    emit_kv(s)
                    emit_rope_q(s, rq[s])

                # phase 2: banded attention + output projection
                for s in range(NST):
                    for kt in range(4 * s, 4 * s + 4):
                        emit_scores(kt, 0)
                    pv0 = emit_pv(s, 0)
                    emit_norm(s, 0, pv0)
                    for kt in range(4 * s, 4 * s + 4):
                        emit_scores(kt, 1)
                    if s >= 1:
                        emit_pso(4 * (s - 1) + 0)
                        emit_pso(4 * (s - 1) + 1)
                    pv1 = emit_pv(s, 1)
                    emit_norm(s, 1, pv1)
                    if s >= 1:
                        emit_pso(4 * (s - 1) + 2)
                        emit_pso(4 * (s - 1) + 3)
                tailp = {}
                for st in range(4 * (NST - 1), 4 * NST):
                    psos = pu.tile([128, 2, 512], F32, tag="pv", name=f"pso{st}")
                    tailp[st] = psos
                    for dx in range(2):
                        nc.tensor.matmul(psos[:, dx, :],
                                         aT[0][:, bass.ts(st, 128)],
                                         wo_sb[:, 0, bass.ts(dx, 512)],
                                         start=True, stop=False)
                for st in range(4 * (NST - 1), 4 * NST):
                    psos = tailp[st]
                    for dx in range(2):
                        nc.tensor.matmul(psos[:, dx, :],
                                         aT[1][:, bass.ts(st, 128)],
                                         wo_sb[:, 1, bass.ts(dx, 512)],
                                         start=False, stop=True)
                    ost = opl.tile([128, 1024], F16, tag="o")
                    if st % 2 == 0:
                        nc.scalar.copy(ost, psos)
                    else:
                        nc.vector.tensor_scalar_mul(ost, psos, 1.0)
                    nc.gpsimd.dma_start(out=o_part[bass.ts(st, 128), :], in_=ost)

    nc.compile()
    return nc


_NC = {}


def _get_nc():
    if "nc" not in _NC:
        _NC["nc"] = _build()
    return _NC["nc"]


def _host_inputs(hidden_states, position_ids, Wq, Wk, Wv, Wo):
    hs = np.asarray(hidden_states, np.float32)
    Wq = np.asarray(Wq, np.float32)
    Wk = np.asarray(Wk, np.float32)
    Wv = np.asarray(Wv, np.float32)
    Wo = np.asarray(Wo, np.float32)

    # sigma: pair-interleaved head-dim order [0, 32, 1, 33, ...]
    sig = np.empty(HD, np.int64)
    sig[0::2] = np.arange(32)
    sig[1::2] = np.arange(32) + 32

    # hsT strip halves, [128, 4, 512] each (partition-contiguous rows)
    hsts = []
    for b in range(B):
        hT = np.ascontiguousarray(hs[b].T).astype(np.float16)      # [D, S]
        halves = []
        for s in range(NST):
            blk = hT[:, 512 * s:512 * (s + 1)].reshape(8, 128, 512).transpose(1, 0, 2)
            halves.append((np.ascontiguousarray(blk[:, 0:4]),
                           np.ascontiguousarray(blk[:, 4:8])))
        hsts.append(halves)

    inv_freq = (1.0 / (THETA ** (np.arange(0, HD, 2, dtype=np.float32) / HD))).astype(np.float32)
    cos2, sin2 = [], []
    for b in range(B):
        pos = np.asarray(position_ids[b]).astype(np.float32)
        freqs = pos[:, None] * inv_freq[None, :]          # [S, 32]
        cosf = np.cos(freqs).T                            # [32, S]
        sinf = np.sin(freqs).T
        cos64 = np.empty((64, S), np.float32)
        cos64[0::2] = cosf
        cos64[1::2] = cosf
        sin64 = np.empty((64, S), np.float32)
        sin64[0::2] = -sinf
        sin64[1::2] = sinf
        cos2.append(np.concatenate([cos64, cos64], axis=0).astype(np.float16))
        sin2.append(np.concatenate([sin64, sin64], axis=0).astype(np.float16))

    p = np.arange(128)[:, None]
    c = np.arange(WIN)[None, :]
    bandmask = np.where((p <= c) & (c < p + WINDOW), 0.0, -30000.0).astype(np.float16)

    in_maps = []
    for core in range(8):
        b, g = divmod(core, 4)
        Wq_g = Wq[256 * g:256 * (g + 1)].reshape(4, HD, D)[:, sig, :].reshape(256, D)
        wqT = np.ascontiguousarray(
            Wq_g.T.reshape(8, 128, 256).transpose(1, 0, 2)).astype(np.float16)
        Wk_g = Wk[64 * g:64 * (g + 1)][sig]
        WKV = np.concatenate([Wv[64 * g:64 * (g + 1)], Wk_g], axis=0)  # [128, D]
        wkvT = np.ascontiguousarray(
            WKV.T.reshape(8, 128, 128).transpose(1, 0, 2)).astype(np.float16)
        woT = np.ascontiguousarray(
            Wo[:, 256 * g:256 * (g + 1)].T.reshape(2, 128, D).transpose(1, 0, 2)).astype(np.float16)
        m = {}
        for s in range(NST):
            m[f"hsTa{s}"] = hsts[b][s][0]
            m[f"hsTb{s}"] = hsts[b][s][1]
        m.update({
            "wqT": wqT, "wkvT": wkvT, "woT": woT,
            "cos2": cos2[b], "sinS2": sin2[b], "bandmask": bandmask,
        })
        in_maps.append(m)
    return in_maps


def run_spmd(hidden_states, attention_mask, position_ids, Wq, Wk, Wv, Wo, **spmd_kwargs):
    nc = _get_nc()
    in_maps = _host_inputs(hidden_states, position_ids, Wq, Wk, Wv, Wo)
    res = run_bass_kernel_spmd(nc, in_maps, list(range(8)), **spmd_kwargs)
    out = np.zeros((B, S, D), np.float32)
    for core in range(8):
        out[core // 4] += np.asarray(res.results[core]["o_part"], np.float32)
    return out, res


def kernel(hidden_states, attention_mask, position_ids, Wq, Wk, Wv, Wo):
    out, _ = run_spmd(hidden_states, attention_mask, position_ids, Wq, Wk, Wv, Wo)
    return out


# revision 4
# speedup vs baseline: 1.0044x; 1.0044x over previous
"""Mimi-style GQA attention (RoPE + 250-wide sliding causal window) on 8 TRN2 NeuronCores.

Sharding: core c handles batch b=c//4 and KV-head group g=c%4 (4 query heads +
1 KV head). Wq/Wk/Wv column-sharded, Wo row-sharded along the head dim; each
core emits a partial [S, D] output (fp16); host sums the 4 partials per batch.

v3 design (vs the 118.7us v2 baseline):
  * All DRAM inputs host-prearranged as [128, X] partition-contiguous blocks:
    every load runs with >=2KB descriptors (the v2 rearranged loads moved at
    ~37GB/s on 512B descriptors, starving the first 25us).
  * hsT streamed strip-major on both HWDGE queues; 16 HAM warm-up matmuls
    cover the stream lead-in so projections start warm at ~8us.
  * Head dims stored pair-interleaved (sigma order [0,32,1,33,...]): RoPE
    rotate-half becomes a within-quadrant partition swap - one vector
    stream_shuffle + two multiplies + add, no PE perm matmuls.
  * The sliding-window mask is applied on the PE as additive -30000 matmuls
    (identity stationary, masked columns only) accumulated into the score
    PSUM before exp: the vector engine leaves the scores->exp->PV chain
    entirely, and the et tiles need no post-exp masking.
  * PV stationaries are [V|ones] / [ones|V] so the softmax denominator lands
    broadcast on complementary partitions for free; normalization is one
    reciprocal_approx_fast + two partition-shift DMAs + two lane-aligned
    multiplies per (strip, head-pair).
  * Two-phase schedule with one unified 4-buf PSUM pool (8 banks): dense
    projection+rope phase, then per-strip scores/exp, PV bursts, norms, with
    the previous strip's output projections as PE filler; channel-split tail.
  * Engine balance: scalar = exp + PSUM staging copies; vector = rope +
    reciprocal + norm + half the output copies; gpsimd = cos-path multiplies
    and bulk DMA dispatch only.
"""
import os
import sys

for _p in ("/opt/trn_rl_repo", "/root/.axon_site/_ro/trn_rl_repo"):
    if os.path.isdir(_p) and _p not in sys.path:
        sys.path.append(_p)

import numpy as np
import concourse.bass as bass
import concourse.mybir as mybir
import concourse.tile as tile
from concourse import bacc
from concourse.bass_utils import run_bass_kernel_spmd
from concourse.masks import make_identity

F32 = mybir.dt.float32
F16 = mybir.dt.float16
AF = mybir.ActivationFunctionType
OP = mybir.AluOpType

B, S, D = 2, 2048, 1024
H, HK, HD = 16, 4, 64
WINDOW = 250
SCALE = 1.0 / np.sqrt(HD)
THETA = 10000.0
NKT = S // 128          # 16 k-tiles
NST = S // 512          # 4 q-strips
WIN = 384               # padded per-k-tile q-window

SHUF_MASK = []
for _i in range(16):
    SHUF_MASK += [2 * _i + 1, 2 * _i]


def _pv_pieces(s):
    """PV pieces for q-strip s: list of (kt, c0, c1) window-column ranges."""
    out = []
    for kt in range(max(0, 4 * s - 2), min(NKT - 1, 4 * s + 3) + 1):
        j0 = 128 * kt
        w = min(WIN, S - j0)
        c_lo = max(0, 512 * s - j0)
        c_hi = min(w, 512 * (s + 1) - j0)
        if c_lo < c_hi:
            out.append((kt, c_lo, c_hi))
    return out


def _build():
    nc = bacc.Bacc(None, target_bir_lowering=False)

    hsta = [nc.declare_dram_parameter(f"hsTa{s}", [128, 4, 512], F16, isOutput=False)
            for s in range(NST)]
    hstb = [nc.declare_dram_parameter(f"hsTb{s}", [128, 4, 512], F16, isOutput=False)
            for s in range(NST)]
    wq = nc.declare_dram_parameter("wqT", [128, 8, 256], F16, isOutput=False)
    wkv = nc.declare_dram_parameter("wkvT", [128, 8, 128], F16, isOutput=False)
    wo = nc.declare_dram_parameter("woT", [128, 2, D], F16, isOutput=False)
    cosd = nc.declare_dram_parameter("cos2", [128, S], F16, isOutput=False)
    sind = nc.declare_dram_parameter("sinS2", [128, S], F16, isOutput=False)
    maskd = nc.declare_dram_parameter("bandmask", [128, WIN], F16, isOutput=False)
    o_part = nc.declare_dram_parameter("o_part", [S, D], F16, isOutput=True)

    with tile.TileContext(nc) as tc:
        with (
            tc.tile_pool(name="persist", bufs=1) as pp,
            tc.tile_pool(name="work", bufs=8) as wkp,
            tc.tile_pool(name="ework", bufs=8) as ep,
            tc.tile_pool(name="norm", bufs=2) as npl,
            tc.tile_pool(name="ost", bufs=3) as opl,
        ):
            # ---- input DMAs (host-prearranged [128, X]: big descriptors) ----
            # scalar HWDGE queue: weights only (small, early, never blocks).
            wq_sb = pp.tile([128, 8, 256], F16, tag="wq")
            wkv_sb = pp.tile([128, 8, 128], F16, tag="wkv")
            nc.scalar.dma_start(out=wq_sb[:, 0:4, :], in_=wq[:, 0:4, :])
            nc.scalar.dma_start(out=wkv_sb, in_=wkv[:, :, :])
            nc.scalar.dma_start(out=wq_sb[:, 4:8, :], in_=wq[:, 4:8, :])
            # sync HWDGE queue: the hsT stream, strip-major; strip 0 in
            # quarter-granularity so the first matmuls gate on ~256KB.
            hta, htb = [], []
            htb_pend = []
            for s in range(NST):
                ta = pp.tile([128, 4, 512], F16, tag=f"hta{s}")
                tb = pp.tile([128, 4, 512], F16, tag=f"htb{s}")
                if s == 0:
                    nc.sync.dma_start(out=ta[:, 0:2, :], in_=hsta[s][:, 0:2, :])
                    nc.sync.dma_start(out=ta[:, 2:4, :], in_=hsta[s][:, 2:4, :])
                    nc.sync.dma_start(out=tb[:, 0:2, :], in_=hstb[s][:, 0:2, :])
                    nc.sync.dma_start(out=tb[:, 2:4, :], in_=hstb[s][:, 2:4, :])
                elif s == 1:
                    nc.sync.dma_start(out=ta, in_=hsta[s][:, :, :])
                    nc.sync.dma_start(out=tb, in_=hstb[s][:, :, :])
                else:
                    # lower halves dispatched mid-phase-1 on the scalar queue
                    nc.sync.dma_start(out=tb, in_=hstb[s][:, :, :])
                hta.append(ta)
                htb.append(tb)
            mask_sb = pp.tile([128, WIN], F16, tag="mask")
            nc.scalar.dma_start(out=mask_sb, in_=maskd[:, :])
            # gpsimd SWDGE queue: aux tensors not needed before ~4us.
            cos_sb = pp.tile([128, S], F16, tag="cos")
            sin_sb = pp.tile([128, S], F16, tag="sin")
            nc.gpsimd.dma_start(out=cos_sb, in_=cosd[:, :])
            nc.gpsimd.dma_start(out=sin_sb, in_=sind[:, :])
            wo_sb = pp.tile([128, 2, D], F16, tag="wo")
            nc.gpsimd.dma_start(out=wo_sb, in_=wo[:, :, :])

            warm = pp.tile([128, 512], F16, tag="warm")
            nc.vector.memset(warm, 0.0)
            ident = pp.tile([64, 64], F16, tag="ident")
            make_identity(nc, ident)
            ident128 = pp.tile([128, 128], F16, tag="id128")
            make_identity(nc, ident128)
            vload = pp.tile([128, NKT, 192], F16, tag="vload")
            nc.gpsimd.memset(vload, 1.0)

            kdup = pp.tile([128, S], F16, tag="kdup")
            qT = [pp.tile([128, S], F16, tag=f"qT{m}", name=f"qT{m}") for m in range(2)]
            aT = [pp.tile([128, S], F16, tag=f"aT{ch}", name=f"aT{ch}") for ch in range(2)]

            with tc.tile_pool(name="pu", bufs=4, space="PSUM") as pu:
                p_sc = p_pv = pu
                expm = {}

                def emit_kv(s):
                    """KV projection + K-RoPE + V transposes. Decoupled from
                    the Q path: scalar only stages rawkv; V^T copies land on
                    vector where they have strip-scale slack."""
                    sl = bass.ts(s, 512)
                    tk = pu.tile([128, 2, 512], F32, tag="pv", name=f"tk{s}")
                    for dt in range(8):
                        h = hta[s][:, dt, :] if dt < 4 else htb[s][:, dt - 4, :]
                        nc.tensor.matmul(tk[:, 0, :], wkv_sb[:, dt, :], h,
                                         start=(dt == 0), stop=(dt == 7))
                    rawkv = wkp.tile([128, 512], F16, tag="rawkv")
                    nc.scalar.copy(rawkv, tk[:, 0, :])
                    rotk = wkp.tile([128, 512], F16, tag="rot")
                    nc.vector.stream_shuffle(rotk[64:128, :], rawkv[64:128, :],
                                             SHUF_MASK)
                    t1k = wkp.tile([128, 512], F16, tag="t1")
                    nc.vector.tensor_tensor(out=t1k[64:128, :], in0=rotk[64:128, :],
                                            in1=sin_sb[64:128, sl], op=OP.mult)
                    t2k = wkp.tile([128, 512], F16, tag="t2")
                    nc.gpsimd.tensor_tensor(out=t2k[64:128, :], in0=rawkv[64:128, :],
                                            in1=cos_sb[64:128, sl], op=OP.mult)
                    nc.vector.tensor_tensor(out=kdup[64:128, sl], in0=t1k[64:128, :],
                                            in1=t2k[64:128, :], op=OP.add)
                    nc.scalar.dma_start(out=kdup[0:64, sl], in_=kdup[64:128, sl])
                    for j in range(4):
                        kt = 4 * s + j
                        tr = tk[:, 1, 32 * j:32 * j + 32].bitcast(F16)
                        nc.tensor.transpose(tr, rawkv[0:64, bass.ts(j, 128)], ident)
                        nc.scalar.copy(vload[:, kt, 64:128], tr)

                def emit_q_mm(s):
                    """Q projection matmuls; PSUM drained by vector copies
                    emitted immediately (they are ready as soon as the
                    accumulation closes, so no FIFO head-of-line risk)."""
                    tq = pu.tile([128, 2, 512], F32, tag="pv", name=f"tq{s}")
                    for dt in range(4):
                        nc.tensor.matmul(tq[:, 0, :], wq_sb[:, dt, 0:128],
                                         hta[s][:, dt, :], start=(dt == 0), stop=False)
                        nc.tensor.matmul(tq[:, 1, :], wq_sb[:, dt, 128:256],
                                         hta[s][:, dt, :], start=(dt == 0), stop=False)
                    for dt in range(4, 8):
                        nc.tensor.matmul(tq[:, 0, :], wq_sb[:, dt, 0:128],
                                         htb[s][:, dt - 4, :], start=False, stop=(dt == 7))
                        nc.tensor.matmul(tq[:, 1, :], wq_sb[:, dt, 128:256],
                                         htb[s][:, dt - 4, :], start=False, stop=(dt == 7))
                    raws = []
                    for m in range(2):
                        raw = wkp.tile([128, 512], F16, tag=f"raw{m}")
                        nc.scalar.copy(raw, tq[:, m, :])
                        raws.append(raw)
                    return raws

                def emit_rope_q(s, raws):
                    sl = bass.ts(s, 512)
                    for m in range(2):
                        raw = raws[m]
                        rot = wkp.tile([128, 512], F16, tag="rot")
                        nc.vector.stream_shuffle(rot, raw, SHUF_MASK)
                        t1 = wkp.tile([128, 512], F16, tag="t1")
                        nc.vector.tensor_tensor(out=t1, in0=rot, in1=sin_sb[:, sl],
                                                op=OP.mult)
                        t2 = wkp.tile([128, 512], F16, tag="t2")
                        nc.gpsimd.tensor_tensor(out=t2, in0=raw,
                                                in1=cos_sb[:, sl], op=OP.mult)
                        nc.vector.tensor_tensor(out=qT[m][:, sl], in0=t1, in1=t2,
                                                op=OP.add)

                def emit_scores(kt, p):
                    j0 = 128 * kt
                    w = min(WIN, S - j0)
                    if p == 0:
                        expm[kt] = ep.tile([128, 4, WIN], F16, tag="e",
                                           name=f"e{kt}")
                    et4 = expm[kt]
                    sc = pu.tile([128, 2, 512], F32, tag="pv", name=f"sc{kt}_{p}")
                    for hh in range(2):
                        half = hh * 64
                        nc.tensor.matmul(
                            sc[:, hh, 0:w],
                            kdup[half:half + 64, bass.ts(kt, 128)],
                            qT[p][half:half + 64, j0:j0 + w],
                            start=True, stop=False)
                    # additive band mask (-30000 outside the window) applied
                    # on the PE: identity stationary, masked columns only.
                    if w == WIN:
                        for hh in range(2):
                            nc.tensor.matmul(sc[:, hh, 0:136], ident128,
                                             mask_sb[:, 0:136],
                                             start=False, stop=False)
                            nc.tensor.matmul(sc[:, hh, 248:384], ident128,
                                             mask_sb[:, 248:384],
                                             start=False, stop=True)
                    else:
                        for hh in range(2):
                            nc.tensor.matmul(sc[:, hh, 0:w], ident128,
                                             mask_sb[:, 0:w],
                                             start=False, stop=True)
                    nc.scalar.activation(et4[:, 2 * p:2 * p + 2, 0:w],
                                         sc[:, :, 0:w], AF.Exp, scale=float(SCALE))

                def emit_pv(s, hp):
                    pv = pu.tile([128, 2, 512], F32, tag="pv", name=f"pv{s}{hp}")
                    pieces = _pv_pieces(s)
                    subs = []
                    cw = 0
                    for kt, c0, c1 in pieces:
                        b = 128 * kt + c0 - 512 * s
                        e = b + (c1 - c0)
                        if b < cw:
                            subs.append((kt, c0, c0 + min(cw, e) - b, b))
                        if e > cw:
                            subs.append((kt, c0 + (max(b, cw) - b), c1, max(b, cw)))
                            cw = e
                    for i, (kt, c0, c1, b) in enumerate(subs):
                        first, last = (i == 0), (i == len(subs) - 1)
                        et4 = expm[kt]
                        nc.tensor.matmul(pv[:, 0, b:b + (c1 - c0)],
                                         vload[:, kt, 64:192],
                                         et4[:, 2 * hp, c0:c1],
                                         start=first, stop=last)
                        nc.tensor.matmul(pv[:, 1, b:b + (c1 - c0)],
                                         vload[:, kt, 0:128],
                                         et4[:, 2 * hp + 1, c0:c1],
                                         start=first, stop=last)
                    return pv

                def emit_norm(s, hp, pv):
                    sl = bass.ts(s, 512)
                    rcp = npl.tile([128, 2, 512], F32, tag="rcp")
                    nc.vector.reciprocal_approx_fast(out=rcp, in_=pv)
                    rcpS = npl.tile([128, 2, 512], F32, tag="rcpS")
                    nc.sync.dma_start(out=rcpS[0:64, 0, :], in_=rcp[64:128, 0, :])
                    nc.sync.dma_start(out=rcpS[64:128, 1, :], in_=rcp[0:64, 1, :])
                    nc.vector.tensor_tensor(out=aT[hp][0:64, sl],
                                            in0=pv[0:64, 0, :],
                                            in1=rcpS[0:64, 0, :], op=OP.mult)
                    nc.vector.tensor_tensor(out=aT[hp][64:128, sl],
                                            in0=pv[64:128, 1, :],
                                            in1=rcpS[64:128, 1, :], op=OP.mult)

                def emit_pso(st, copy_eng=None):
                    psos = pu.tile([128, 2, 512], F32, tag="pv", name=f"pso{st}")
                    for ch in range(2):
                        for dx in range(2):
                            nc.tensor.matmul(psos[:, dx, :],
                                             aT[ch][:, bass.ts(st, 128)],
                                             wo_sb[:, ch, bass.ts(dx, 512)],
                                             start=(ch == 0), stop=(ch == 1))
                    ost = opl.tile([128, 1024], F16, tag="o")
                    eng = copy_eng or ("scalar" if st % 2 == 0 else "vector")
                    if eng == "scalar":
                        nc.scalar.copy(ost, psos)
                    else:
                        nc.vector.tensor_scalar_mul(ost, psos, 1.0)
                    nc.sync.dma_start(out=o_part[bass.ts(st, 128), :], in_=ost)

                # ---- master schedule ----
                wps = pu.tile([128, 2, 512], F32, tag="pv", name="warm")
                for _w in range(16):
                    nc.tensor.matmul(wps[:, _w % 2, :], warm[:, 0:128], warm,
                                     start=True, stop=True)

                # phase 1: dense projections + rope, strip-streamed with DMA
                rq = {}
                for s in range(NST):
                    if s == 2:
                        nc.scalar.dma_start(out=hta[2], in_=hsta[2][:, :, :])
                        nc.scalar.dma_start(out=hta[3], in_=hsta[3][:, :, :])
                    rq[s] = emit_q_mm(s)
                    emit_kv(s)
                    emit_rope_q(s, rq[s])

                # phase 2: banded attention + output projection
                for s in range(NST):
                    for kt in range(4 * s, 4 * s + 4):
                        emit_scores(kt, 0)
                    pv0 = emit_pv(s, 0)
                    emit_norm(s, 0, pv0)
                    for kt in range(4 * s, 4 * s + 4):
                        emit_scores(kt, 1)
                    if s >= 1:
                        emit_pso(4 * (s - 1) + 0)
                        emit_pso(4 * (s - 1) + 1)
                    pv1 = emit_pv(s, 1)
                    emit_norm(s, 1, pv1)
                    if s >= 1:
                        emit_pso(4 * (s - 1) + 2)
                        emit_pso(4 * (s - 1) + 3)
                tailp = {}
                for st in range(4 * (NST - 1), 4 * NST):
                    psos = pu.tile([128, 2, 512], F32, tag="pv", name=f"pso{st}")
                    tailp[st] = psos
                    for dx in range(2):
                        nc.tensor.matmul(psos[:, dx, :],
                                         aT[0][:, bass.ts(st, 128)],
                                         wo_sb[:, 0, bass.ts(dx, 512)],
                                         start=True, stop=False)
                for st in range(4 * (NST - 1), 4 * NST):
                    psos = tailp[st]
                    for dx in range(2):
                        nc.tensor.matmul(psos[:, dx, :],
                                         aT[1][:, bass.ts(st, 128)],
                                         wo_sb[:, 1, bass.ts(dx, 512)],
                                         start=False, stop=True)
                    ost = opl.tile([128, 1024], F16, tag="o")
                    if st % 2 == 0:
                        nc.scalar.copy(ost, psos)
                    else:
                        nc.vector.tensor_scalar_mul(ost, psos, 1.0)
                    nc.sync.dma_start(out=o_part[bass.ts(st, 128), :], in_=ost)

    nc.compile()
    return nc


_NC = {}


def _get_nc():
    if "nc" not in _NC:
        _NC["nc"] = _build()
    return _NC["nc"]


def _host_inputs(hidden_states, position_ids, Wq, Wk, Wv, Wo):
    hs = np.asarray(hidden_states, np.float32)
    Wq = np.asarray(Wq, np.float32)
    Wk = np.asarray(Wk, np.float32)
    Wv = np.asarray(Wv, np.float32)
    Wo = np.asarray(Wo, np.float32)

    # sigma: pair-interleaved head-dim order [0, 32, 1, 33, ...]
    sig = np.empty(HD, np.int64)
    sig[0::2] = np.arange(32)
    sig[1::2] = np.arange(32) + 32

    # hsT strip halves, [128, 4, 512] each (partition-contiguous rows)
    hsts = []
    for b in range(B):
        hT = np.ascontiguousarray(hs[b].T).astype(np.float16)      # [D, S]
        halves = []
        for s in range(NST):
            blk = hT[:, 512 * s:512 * (s + 1)].reshape(8, 128, 512).transpose(1, 0, 2)
            halves.append((np.ascontiguousarray(blk[:, 0:4]),
                           np.ascontiguousarray(blk[:, 4:8])))
        hsts.append(halves)

    inv_freq = (1.0 / (THETA ** (np.arange(0, HD, 2, dtype=np.float32) / HD))).astype(np.float32)
    cos2, sin2 = [], []
    for b in range(B):
        pos = np.asarray(position_ids[b]).astype(np.float32)
        freqs = pos[:, None] * inv_freq[None, :]          # [S, 32]
        cosf = np.cos(freqs).T                            # [32, S]
        sinf = np.sin(freqs).T
        cos64 = np.empty((64, S), np.float32)
        cos64[0::2] = cosf
        cos64[1::2] = cosf
        sin64 = np.empty((64, S), np.float32)
        sin64[0::2] = -sinf
        sin64[1::2] = sinf
        cos2.append(np.concatenate([cos64, cos64], axis=0).astype(np.float16))
        sin2.append(np.concatenate([sin64, sin64], axis=0).astype(np.float16))

    p = np.arange(128)[:, None]
    c = np.arange(WIN)[None, :]
    bandmask = np.where((p <= c) & (c < p + WINDOW), 0.0, -30000.0).astype(np.float16)

    in_maps = []
    for core in range(8):
        b, g = divmod(core, 4)
        Wq_g = Wq[256 * g:256 * (g + 1)].reshape(4, HD, D)[:, sig, :].reshape(256, D)
        wqT = np.ascontiguousarray(
            Wq_g.T.reshape(8, 128, 256).transpose(1, 0, 2)).astype(np.float16)
        Wk_g = Wk[64 * g:64 * (g + 1)][sig]
        WKV = np.concatenate([Wv[64 * g:64 * (g + 1)], Wk_g], axis=0)  # [128, D]
        wkvT = np.ascontiguousarray(
            WKV.T.reshape(8, 128, 128).transpose(1, 0, 2)).astype(np.float16)
        woT = np.ascontiguousarray(
            Wo[:, 256 * g:256 * (g + 1)].T.reshape(2, 128, D).transpose(1, 0, 2)).astype(np.float16)
        m = {}
        for s in range(NST):
            m[f"hsTa{s}"] = hsts[b][s][0]
            m[f"hsTb{s}"] = hsts[b][s][1]
        m.update({
            "wqT": wqT, "wkvT": wkvT, "woT": woT,
            "cos2": cos2[b], "sinS2": sin2[b], "bandmask": bandmask,
        })
        in_maps.append(m)
    return in_maps


def run_spmd(hidden_states, attention_mask, position_ids, Wq, Wk, Wv, Wo, **spmd_kwargs):
    nc = _get_nc()
    in_maps = _host_inputs(hidden_states, position_ids, Wq, Wk, Wv, Wo)
    res = run_bass_kernel_spmd(nc, in_maps, list(range(8)), **spmd_kwargs)
    out = np.zeros((B, S, D), np.float32)
    for core in range(8):
        out[core // 4] += np.asarray(res.results[core]["o_part"], np.float32)
    return out, res


def kernel(hidden_states, attention_mask, position_ids, Wq, Wk, Wv, Wo):
    out, _ = run_spmd(hidden_states, attention_mask, position_ids, Wq, Wk, Wv, Wo)
    return out
